# revision 12
# baseline (speedup 1.0000x reference)
"""Distributed Trainium2 kernel for 16-head causal attention (B=4, T=2048, D=1024).

Sharding (Megatron-style, per the hint): 8 cores = 4 batch pairs.
Core c handles batch c//2 and head-group c%2 (8 heads = 512 of D).
Each core computes its QKV projections (transposed layout), causal
attention for its 8 heads (scores computed as S^T = K Q^T so the AV
matmul needs no transposes; softmax needs no max-subtraction since
scores are ~N(0,1); the denominator comes for free from a ones-column
appended to V), then its partial output projection.  The two cores of a
batch pair combine bf16 partials with pairwise ReduceScatters (two
chunks, overlapping the output projection); the host concatenates the
row-quarters.

Performance structure (v2):
- Score matmuls for the two local heads (K=64 each, partitions 0:64 /
  64:128) are emitted as adjacent instruction pairs targeting different
  PSUM banks: the PE runs disjoint-row-group matmuls CONCURRENTLY
  (trace-measured dstart ~3-30ns), so a head-pair costs one matmul slot.
  v1 already got this in the full-j region; v2 packs the diagonal
  blocks the same way ([h0 | h64] at bank-aligned column offsets of a
  shared tile).
- Full-region scores use one [128,1024] psum tile per j-block
  ([h0|h64]), exp'd in a single ACT instruction; AV matmuls trail the
  QK stream by 2 j-blocks so the ACT exp latency and the psum-recycle
  wait are hidden behind independent PE work.
- Out-projection work is chopped into per-i-block "filler" quanta and
  injected between attention j-blocks, keeping the PE busy through the
  ACT-bound late chunks.
- Softmax normalization is deferred off the AV critical path (DRAM
  round-trip partition-broadcast of the reciprocal denominators,
  multiply deferred to the next m's QK stream), as in v1.
- RS-output forward DMAs + fence reads live on the GPSIMD queue (with
  the collective triggers) so a slow ReduceScatter can never
  head-of-line block the sync queue that carries the normalization
  round-trips (v1 lost ~8us + a HAM re-throttle to this).
- QK projection PSUM->SBUF copies run on GPSIMD (idle) instead of DVE
  (41% busy), halving the copy serialization in the projection phase.
- A fence (read one row of each output back + DVE copy) keeps the NEFF
  epilogue from signalling done while the output forward DMAs are in
  flight.
"""

import sys

sys.path.insert(0, "/opt/trn_rl_repo")

import numpy as np
import ml_dtypes

import concourse.bass as bass
import concourse.mybir as mybir
import concourse.tile as tile
from concourse import bacc
from concourse.bass_utils import run_bass_kernel_spmd

BF16 = mybir.dt.bfloat16
F32 = mybir.dt.float32
P = 128
D_MODEL = 1024
D_LOCAL = 512  # 8 heads x 64 per core
H_LOCAL = 8
HD = 64
N_CORES = 8
EXP_SCALE = 0.125  # 1/sqrt(64)
# ReduceScatter chunks as (start_i_block, n_i_blocks): early chunks fire
# mid-kernel and their RS hides under attention compute; the last covers
# exactly the final i-chunk so only ONE collective (whose ~15us cost is
# fixed-dominated, so splitting it doesn't pay) trails the last out-proj.
CHUNKS = [(0, 6), (6, 6), (12, 4)]
NCH = len(CHUNKS)

Exp = mybir.ActivationFunctionType.Exp
Mult = mybir.AluOpType.mult


def build_nc(T, debug_taps=False):
    """Build the SPMD Bass graph (identical on all 8 cores)."""
    assert T % 512 == 0
    TB = T // 128  # t-blocks
    TC = T // 512  # i-chunks

    nc = bacc.Bacc(None, target_bir_lowering=False, debug=False,
                   num_devices=N_CORES)

    xT_d = nc.dram_tensor("xT", [D_MODEL, T], BF16, kind="ExternalInput")
    wqT_d = nc.dram_tensor("wqT", [D_MODEL, D_LOCAL], BF16, kind="ExternalInput")
    wkT_d = nc.dram_tensor("wkT", [D_MODEL, D_LOCAL], BF16, kind="ExternalInput")
    wvT_d = nc.dram_tensor("wvT", [D_MODEL, D_LOCAL], BF16, kind="ExternalInput")
    woT_d = nc.dram_tensor("woT", [D_LOCAL, D_MODEL], BF16, kind="ExternalInput")

    # chunked pairwise ReduceScatter (bf16).  The collective may not write
    # IO tensors, so rs_out is Shared scratch (fast HBM-HBM path) and a
    # plain DMA forwards each chunk to its bf16 output tensor — no compute
    # engine touches the RS results, so nothing can stall on a slow RS.
    rs_in = [nc.dram_tensor(f"rs_in{c}", [n * 128, D_MODEL], BF16)
             for c, (s, n) in enumerate(CHUNKS)]
    rs_out = [nc.dram_tensor(f"rs_out{c}", [n * 64, D_MODEL], BF16)
              for c, (s, n) in enumerate(CHUNKS)]
    out_d = [nc.dram_tensor(f"out{c}", [n * 64, D_MODEL], BF16,
                            kind="ExternalOutput")
             for c, (s, n) in enumerate(CHUNKS)]

    # Upper-triangular (incl. diagonal) multiplicative mask for the
    # transposed-score layout: e^T[j, i] valid iff i >= j.
    tri_np = (np.arange(128)[None, :] >= np.arange(128)[:, None])
    tri_d = nc.inline_tensor(tri_np.astype(ml_dtypes.bfloat16), name="tri")
    ones_d = nc.inline_tensor(np.ones((P, P), dtype=ml_dtypes.bfloat16),
                              name="onesblk")
    # head-half selector columns for the last-m fast normalization (cols
    # 0:128 select partition rows 0:64, cols 128:256 the rest)
    m2_np = np.zeros((1, 256), dtype=ml_dtypes.bfloat16)
    m2_np[0, 0:64] = 1
    m2_np[0, 192:256] = 1
    mask2_d = nc.inline_tensor(m2_np, name="mask2")

    with tile.TileContext(nc) as tc:
        with (
            tc.tile_pool(name="persist", bufs=1) as wpool,
            tc.tile_pool(name="efull", bufs=9) as epool,
            tc.tile_pool(name="ediag", bufs=6) as edpool,
            tc.tile_pool(name="small", bufs=3) as spool,
            tc.tile_pool(name="osb", bufs=3) as opool,
            tc.tile_pool(name="psum", bufs=3, space="PSUM") as psum,
            tc.tile_pool(name="psum_av", bufs=2, space="PSUM") as psum_av,
        ):
            tri_sb = wpool.tile([P, P], BF16, tag="tri")
            nc.sync.dma_start(tri_sb[:], tri_d.ap())
            ones_sb = wpool.tile([P, P], BF16, tag="ones")
            nc.sync.dma_start(ones_sb[:], ones_d.ap())
            mask2_sb = wpool.tile([1, 2 * P], BF16, tag="mask2")
            nc.sync.dma_start(mask2_sb[:], mask2_d.ap())

            xT_sb = wpool.tile([P, 8, T], BF16, tag="xT")
            wq_sb = wpool.tile([P, 8, D_LOCAL], BF16, tag="wq")
            wk_sb = wpool.tile([P, 8, D_LOCAL], BF16, tag="wk")
            wv_sb = wpool.tile([P, 8, D_LOCAL], BF16, tag="wv")
            wo_sb = wpool.tile([P, 4, D_MODEL], BF16, tag="wo")
            qT_sb = wpool.tile([P, 4, T], BF16, tag="qT")
            kT_sb = wpool.tile([P, 4, T], BF16, tag="kT")
            # v with a ones-column appended per head (65 cols per head)
            v_sb = wpool.tile([P, TB, H_LOCAL * 65], BF16, tag="v")
            attnT_sb = wpool.tile([P, 4, T], BF16, tag="attnT")

            # input loads split across the sync and gpsimd DMA queues and
            # t-sliced so the first v-projection (which needs all 8 x
            # slots but only t<512) can start at ~5us instead of waiting
            # ~22us for the full 8MB of inputs.  x/wq/wk/wv use a (p o)
            # contraction-block mapping — partition p holds D-rows
            # 8p..8p+7 — so a t-half load reads 2KB per (partition, slot)
            # line (full-burst efficient).  wo keeps (o p): its partition
            # mapping must match attnT's, which is fixed by the AV output.
            wq_r = wqT_d.ap().rearrange("(p o) d -> p o d", p=P)
            wk_r = wkT_d.ap().rearrange("(p o) d -> p o d", p=P)
            wv_r = wvT_d.ap().rearrange("(p o) d -> p o d", p=P)
            wo_r = woT_d.ap().rearrange("(o p) e -> p o e", p=P)
            xT_r = xT_d.ap().rearrange("(p o) t -> p o t", p=P)
            Th = T // 2
            nc.sync.dma_start(xT_sb[:, 0:4, 0:Th], xT_r[:, 0:4, 0:Th])
            nc.gpsimd.dma_start(wv_sb[:], wv_r)
            nc.gpsimd.dma_start(xT_sb[:, 4:8, 0:Th], xT_r[:, 4:8, 0:Th])
            nc.sync.dma_start(xT_sb[:, 0:4, Th:T], xT_r[:, 0:4, Th:T])
            nc.gpsimd.dma_start(wq_sb[:], wq_r)
            nc.gpsimd.dma_start(wk_sb[:], wk_r)
            nc.gpsimd.dma_start(xT_sb[:, 4:8, Th:T], xT_r[:, 4:8, Th:T])
            nc.sync.dma_start(wo_sb[:], wo_r)

            # ones columns of v (col 64 of each head's 65-wide slot):
            # one strided DVE copy from a dense const block
            v_view = v_sb[:].rearrange("p t (h c) -> p t h c", c=65)
            nc.vector.tensor_copy(
                v_view[:, :, :, 64:65],
                ones_sb[:, 0:TB * H_LOCAL].rearrange(
                    "p (t h o) -> p t h o", h=H_LOCAL, o=1),
            )

            # ---- projection emitters (interleaved into the chunk loop) ----
            def emit_qkproj(m):
                # q^T, k^T block m: [d, t] layout (lhsT = W^T, rhs = x^T).
                # PSUM->SBUF copies go on ACT (idle during the projection
                # phase; GPSIMD cannot read PSUM) so they don't serialize
                # with the v copies on DVE.
                for w_sb, dst in ((wq_sb, qT_sb), (wk_sb, kT_sb)):
                    for t0 in range(0, T, 1024):
                        wdt = min(1024, T - t0)
                        ps = psum.tile([P, 1024], F32, tag="mm2")
                        for k in range(8):
                            for half in range(wdt // 512):
                                hs = slice(half * 512, half * 512 + 512)
                                nc.tensor.matmul(
                                    ps[:, hs],
                                    lhsT=w_sb[:, k, m * 128:(m + 1) * 128],
                                    rhs=xT_sb[:, k, t0 + half * 512:
                                              t0 + half * 512 + 512],
                                    start=(k == 0), stop=(k == 7),
                                )
                        nc.scalar.copy(dst[:, m, t0:t0 + wdt],
                                       ps[:, 0:wdt])

            def emit_vproj(tb_lo, tb_hi):
                # v blocks: [t, d] layout (lhsT = x^T, rhs = W^T), scattered
                # into the 65-stride per-head slots; 2 t-blocks per psum
                for tb0 in range(tb_lo, tb_hi, 2):
                    ps = psum.tile([P, 1024], F32, tag="mm2")
                    for half in range(2):
                        tb = tb0 + half
                        hs = slice(half * 512, half * 512 + 512)
                        for k in range(8):
                            nc.tensor.matmul(
                                ps[:, hs],
                                lhsT=xT_sb[:, k, tb * 128:(tb + 1) * 128],
                                rhs=wv_sb[:, k, :],
                                start=(k == 0), stop=(k == 7),
                            )
                    nc.vector.tensor_copy(
                        v_view[:, tb0:tb0 + 2, :, 0:64],
                        ps[:].rearrange("p (t h c) -> p t h c", t=2, c=64),
                    )

            # ---- out-projection filler quanta: one i-block each, injected
            # between attention j-blocks so the PE stays busy while ACT
            # grinds through the exps.  RS for a chunk fires from the
            # closure that emits its last i-block. ----
            fillers = []

            def make_outproj_filler(ib):
                def f():
                    ch = next(c for c, (s, n) in enumerate(CHUNKS)
                              if s <= ib < s + n)
                    rbase = (ib - CHUNKS[ch][0]) * 128
                    ps = psum.tile([P, 1024], F32, tag="mm2")
                    for dm in range(4):
                        for half in range(2):
                            hs = slice(half * 512, half * 512 + 512)
                            nc.tensor.matmul(
                                ps[:, hs],
                                lhsT=attnT_sb[:, dm, ib * 128:(ib + 1) * 128],
                                rhs=wo_sb[:, dm, half * 512:half * 512 + 512],
                                start=(dm == 0), stop=(dm == 3),
                            )
                    o = opool.tile([P, 1024], BF16, tag="o")
                    nc.vector.tensor_copy(o[:], ps[:])
                    nc.sync.dma_start(
                        rs_in[ch].ap()[rbase:rbase + 128, :], o[:])
                    # fire the ReduceScatter as soon as its rows all exist
                    if ib == CHUNKS[ch][0] + CHUNKS[ch][1] - 1:
                        nc.gpsimd.collective_compute(
                            "ReduceScatter",
                            mybir.AluOpType.add,
                            replica_groups=[[0, 1], [2, 3], [4, 5], [6, 7]],
                            ins=[rs_in[ch].ap().opt()],
                            outs=[rs_out[ch].ap().opt()],
                        )
                return f

            def pop_filler():
                if fillers:
                    fillers.pop(0)()

            # ---- deferred normalization finalize: broadcast the
            # reciprocal denominators across partitions with two K=1 PE
            # matmuls against the head-half selector (rb[p,i] =
            # recb[h(p)*W + i]), then multiply attnT in place on DVE.
            # Deferring this to the next m's QK stream hides the DVE
            # reciprocal-chain latency from the PE queue.  (v1 used a
            # DRAM round-trip DMA broadcast here; its sync-queue DMAs
            # starved for ~11us whenever a ReduceScatter's data phase
            # was in flight, stalling the PE.) ----
            norm_pending = []

            def emit_norm_finalize():
                while norm_pending:
                    recb, m, i0, W = norm_pending.pop(0)
                    rbp = psum.tile([P, 1024], F32, tag="mm2", name="rbp")
                    nc.tensor.matmul(rbp[:, 0:W], lhsT=mask2_sb[0:1, 0:128],
                                     rhs=recb[0:1, 0:W], start=True,
                                     stop=False, skip_group_check=True)
                    nc.tensor.matmul(rbp[:, 0:W],
                                     lhsT=mask2_sb[0:1, 128:256],
                                     rhs=recb[0:1, W:2 * W], start=False,
                                     stop=True, skip_group_check=True)
                    nc.vector.tensor_tensor(
                        attnT_sb[:, m, i0:i0 + W],
                        attnT_sb[:, m, i0:i0 + W], rbp[:, 0:W], Mult)

            # ---- per-(i0, m) attention emitter (W == 512).
            # fast_norm: normalize inline via a PE partition-broadcast
            # (used only for the very last m, where there is no following
            # QK block to hide the DRAM round-trip behind and the PE would
            # otherwise idle ~5us before the final out-projections) ----
            def emit_attn(i0, W, m, den, fast_norm=False, pre_norm_hook=None):
                assert W == 512
                nfull = i0 // 128  # full (non-diagonal) j-blocks
                rows_of = (slice(0, 64), slice(64, 128))
                vslot_of = tuple(slice((2 * m + h) * 65, (2 * m + h + 1) * 65)
                                 for h in (0, 1))
                avps = [psum_av.tile([P, 512], F32, tag="av", name=f"av{h}")
                        for h in (0, 1)]
                e_full = []
                pop_filler()  # entry slot: independent PE work first
                fin_jb = min(2, nfull - 1)  # late enough that recb is ready

                def emit_av_full(jb):
                    e = e_full[jb]
                    for h_loc in (0, 1):
                        nc.tensor.matmul(
                            avps[h_loc][0:65, 0:W],
                            lhsT=v_sb[:, jb, vslot_of[h_loc]],
                            rhs=e[:, h_loc * 512:h_loc * 512 + W],
                            start=(jb == 0), stop=False,
                            skip_group_check=True,
                        )

                # full tiles: S^T = K Q^T per j-block into one [128,1024]
                # tile as [h0 | h64] — the two K=64 matmuls hit disjoint
                # row-groups AND different psum banks, so the PE runs them
                # concurrently.  One exp instruction covers both heads.
                # AV trails by 2 j-blocks to hide the exp latency.
                av_next = 0
                for jb in range(nfull):
                    ps = psum.tile([P, 1024], F32, tag="mm2")
                    for h_loc in (0, 1):
                        nc.tensor.matmul(
                            ps[:, h_loc * 512:h_loc * 512 + W],
                            lhsT=kT_sb[rows_of[h_loc], m,
                                       jb * 128:(jb + 1) * 128],
                            rhs=qT_sb[rows_of[h_loc], m, i0:i0 + W],
                            start=True, stop=True,
                            skip_group_check=True,
                        )
                    e = epool.tile([P, 1024], BF16, tag="ef2")
                    nc.scalar.activation(e[:], ps[:], Exp, scale=EXP_SCALE)
                    e_full.append(e)
                    if jb == fin_jb:
                        # previous m's normalization: its PE broadcast and
                        # DVE multiply trail this m's QK stream, hiding
                        # the reciprocal-chain latency
                        emit_norm_finalize()
                    if jb >= 2:
                        emit_av_full(av_next)
                        av_next += 1
                    if jb in (3, 6, 9):
                        pop_filler()

                # diagonal region: 4 j-blocks (nfull+r covers i-cols
                # [r*128, W)) packed [h0 | h64] at bank-aligned offsets of
                # 3 shared tiles so each h0/h64 pair runs concurrently:
                #   X: r0 h0@0(512)   h64@512(512)   exp [0:1024]
                #   Y: r1 h0@0(384)   h64@512(384)   exp [0:896]
                #   Z: r2 h0@0(256)   h64@512(256)
                #      r3 h0@256(128) h64@768(128)   exp [0:896]
                # (Y/Z cols 384:512 are never written; the exp of that
                # stale psum lands in e-tile cols nothing ever reads.)
                packing = [(0, 0, 0), (1, 1, 0), (2, 2, 0), (3, 2, 256)]
                dwidths = (1024, 896, 896)
                pds = [psum.tile([P, 1024], F32, tag="mm2", name=f"pd{ti}")
                       for ti in range(3)]
                for r, ti, boff in packing:
                    jb = nfull + r
                    width = W - r * 128
                    for h_loc in (0, 1):
                        off = boff + h_loc * 512
                        nc.tensor.matmul(
                            pds[ti][:, off:off + width],
                            lhsT=kT_sb[rows_of[h_loc], m,
                                       jb * 128:(jb + 1) * 128],
                            rhs=qT_sb[rows_of[h_loc], m,
                                      i0 + r * 128:i0 + W],
                            start=(off % 512 == 0), stop=True,
                            skip_group_check=True,
                        )
                    if r < 3:
                        pop_filler()
                    if r == 1 and nfull >= 2:
                        emit_av_full(av_next)
                        av_next += 1
                if nfull == 0:
                    emit_norm_finalize()  # no full QK pairs to trail
                eds = []
                for ti in range(3):
                    ed = edpool.tile([P, dwidths[ti]], BF16, tag="ed")
                    nc.scalar.activation(ed[:], pds[ti][:, 0:dwidths[ti]],
                                         Exp, scale=EXP_SCALE)
                    eds.append(ed)
                    if ti == 0 and nfull >= 3:
                        emit_av_full(av_next)
                        av_next += 1
                # drain remaining full-region AVs
                while av_next < nfull:
                    emit_av_full(av_next)
                    av_next += 1
                # triangular mask on the leading 128 cols of each r
                e_diag = {}
                for r, ti, boff in packing:
                    for h_loc in (0, 1):
                        off = boff + h_loc * 512
                        nc.vector.tensor_tensor(
                            eds[ti][:, off:off + 128],
                            eds[ti][:, off:off + 128], tri_sb[:], Mult)
                        e_diag[(h_loc, r)] = (eds[ti], off)

                # AV over the diagonal region: psum[0:64] = unnormalized
                # attn^T, psum[64] = denom.  The attn^T copies are
                # deferred until after both den copies + reciprocal, so
                # the reciprocal chain starts the moment the last AV
                # drains and recb is ready early for the deferred
                # finalize.
                for h_loc in (0, 1):
                    for r in range(4):
                        ed, base = e_diag[(h_loc, r)]
                        width = W - r * 128
                        nc.tensor.matmul(
                            avps[h_loc][0:65, r * 128:W],
                            lhsT=v_sb[:, nfull + r, vslot_of[h_loc]],
                            rhs=ed[:, base:base + width],
                            # start=True pends the WHOLE psum bank (2KB
                            # zero-region): only the tile's very first
                            # matmul may set it
                            start=(nfull == 0 and r == 0),
                            stop=(r == 3),
                            skip_group_check=True,
                        )
                    # stash denominator row
                    # (DVE operands may sit at different partition bases)
                    nc.vector.tensor_copy(
                        den[0:1, h_loc * W:h_loc * W + W],
                        avps[h_loc][64:65, 0:W])

                # per-(i0, m) softmax normalization, phase 1: reciprocal
                # of the two denominator rows (single-partition DVE ops),
                # then stash the unnormalized attn^T.  The partition
                # broadcast + in-place multiply are deferred until the
                # next m's QK matmuls are enqueued (emit_norm_finalize).
                rec = spool.tile([P, 1024], F32, tag="rec")
                nc.vector.reciprocal_approx_fast(rec[0:1, 0:2 * W],
                                                 den[0:1, 0:2 * W])
                recb = spool.tile([P, 1024], BF16, tag="recb")
                nc.vector.tensor_copy(recb[0:1, 0:2 * W], rec[0:1, 0:2 * W])
                for h_loc in (0, 1):
                    nc.vector.tensor_copy(
                        attnT_sb[h_loc * 64:h_loc * 64 + 64, m, i0:i0 + W],
                        avps[h_loc][0:64, 0:W])
                if fast_norm:
                    if pre_norm_hook is not None:
                        # independent PE work (out-proj partials over the
                        # already-normalized m's) emitted ahead of the
                        # broadcast matmuls so the Tensor queue doesn't
                        # idle on the Vector reciprocal chain
                        pre_norm_hook()
                    rbp = psum.tile([P, 1024], F32, tag="mm2")
                    nc.tensor.matmul(rbp[:, 0:W], lhsT=mask2_sb[0:1, 0:128],
                                     rhs=recb[0:1, 0:W], start=True,
                                     stop=False, skip_group_check=True)
                    nc.tensor.matmul(rbp[:, 0:W],
                                     lhsT=mask2_sb[0:1, 128:256],
                                     rhs=recb[0:1, W:2 * W], start=False,
                                     stop=True, skip_group_check=True)
                    nc.vector.tensor_tensor(
                        attnT_sb[:, m, i0:i0 + W],
                        attnT_sb[:, m, i0:i0 + W], rbp[:, 0:W], Mult)
                    return
                norm_pending.append((recb, m, i0, W))

            # ---- chunk schedule: interleave projections, attention and
            # out-proj fillers so PE always has independent matmuls. ----
            # held psums for the last chunk's ib12/13 out-proj partials,
            # emitted inside the final emit_attn via pre_norm_hook
            held = {}

            def prenorm_partials():
                for ib in (12, 13):
                    ps = psum.tile([P, 1024], F32, tag="mm2",
                                   name=f"op{ib}")
                    for dm in range(3):
                        for half in range(2):
                            hs = slice(half * 512, half * 512 + 512)
                            nc.tensor.matmul(
                                ps[:, hs],
                                lhsT=attnT_sb[:, dm,
                                              ib * 128:(ib + 1) * 128],
                                rhs=wo_sb[:, dm, hs],
                                start=(dm == 0), stop=False,
                                skip_group_check=True,
                            )
                    held[ib] = ps

            def make_qkproj_filler(m, w_sb, dst, t0):
                def f():
                    ps = psum.tile([P, 1024], F32, tag="mm2")
                    for k in range(8):
                        for half in range(2):
                            hs = slice(half * 512, half * 512 + 512)
                            nc.tensor.matmul(
                                ps[:, hs],
                                lhsT=w_sb[:, k, m * 128:(m + 1) * 128],
                                rhs=xT_sb[:, k, t0 + half * 512:
                                          t0 + half * 512 + 512],
                                start=(k == 0), stop=(k == 7),
                            )
                    nc.scalar.copy(dst[:, m, t0:t0 + 1024], ps[:])
                return f

            emit_vproj(0, min(4, TB))
            emit_qkproj(0)
            for m in range(4):
                if m < 3:
                    # next m's q/k projection as 4 filler quanta consumed
                    # inside this m's attention emission
                    fillers.extend(
                        make_qkproj_filler(m + 1, w_sb, dst, t0)
                        for t0 in range(0, T, 1024)
                        for w_sb, dst in ((wq_sb, qT_sb), (wk_sb, kT_sb)))
                den = spool.tile([P, 1024], F32, tag="den")
                emit_attn(0, 512, m, den)
            pending = [[0, 1, 2, 3]]
            for ic in range(1, TC):
                emit_vproj(4 * ic, 4 * ic + 4)
                # the previous chunk's m=3 norm finalizes inside this
                # chunk's m=0 first QK pair; its out-proj blocks become
                # fillers right after that, consumed across m=0..3
                for m in range(4):
                    den = spool.tile([P, 1024], F32, tag="den")
                    fast = (ic == TC - 1 and m == 3)
                    if m == 1 and pending:
                        fillers.extend(make_outproj_filler(ib)
                                       for ib in pending.pop(0))
                    emit_attn(ic * 512, 512, m, den, fast_norm=fast,
                              pre_norm_hook=prenorm_partials if fast
                              else None)
                pending.append([4 * ic, 4 * ic + 1, 4 * ic + 2, 4 * ic + 3])
            emit_norm_finalize()  # no-op unless a norm is still pending
            while fillers:
                fillers.pop(0)()
            while pending[:-1]:
                for ib in pending.pop(0):
                    make_outproj_filler(ib)()
            # fused final tail: finish ib12/13 (dm=3 after the last-m
            # normalization) on the held psums, then ib14/15 + the RS
            for ib in (12, 13):
                ps = held[ib]
                for half in range(2):
                    hs = slice(half * 512, half * 512 + 512)
                    nc.tensor.matmul(
                        ps[:, hs],
                        lhsT=attnT_sb[:, 3, ib * 128:(ib + 1) * 128],
                        rhs=wo_sb[:, 3, hs],
                        start=False, stop=True, skip_group_check=True,
                    )
                o = opool.tile([P, 1024], BF16, tag="o")
                nc.vector.tensor_copy(o[:], ps[:])
                nc.sync.dma_start(
                    rs_in[NCH - 1].ap()[(ib - 12) * 128:
                                        (ib - 12) * 128 + 128, :], o[:])
            for ib in (14, 15):
                make_outproj_filler(ib)()

            # forward RS results to the output tensors (pure DRAM-to-DRAM
            # DMAs, each waiting only on its own collective) on the GPSIMD
            # queue — the sync queue carries the normalization round-trips
            # and must never wait behind a slow collective.  Each fence
            # read stalls the gpsimd queue until its forward's completion
            # semaphore fires, so the NEFF epilogue cannot signal done
            # while an output copy is still in flight (without this, a
            # fresh-load run intermittently returned partial outputs).
            chk = opool.tile([P, 4 * 16], BF16, tag="chk")
            for c in range(NCH):
                nc.gpsimd.dma_start(out_d[c].ap(), rs_out[c].ap())
                nc.gpsimd.dma_start(chk[0:1, c * 16:(c + 1) * 16],
                                    out_d[c].ap()[0:1, 0:16])
            chk2 = opool.tile([P, 4 * 16], BF16, tag="chk2")
            nc.vector.tensor_copy(chk2[0:1, :], chk[0:1, :])

            if debug_taps:
                qT_t = nc.dram_tensor("dbg_qT", [P, 4, T], BF16)
                kT_t = nc.dram_tensor("dbg_kT", [P, 4, T], BF16)
                v_t = nc.dram_tensor("dbg_v", [P, TB, H_LOCAL * 65], BF16)
                at_t = nc.dram_tensor("dbg_attnT", [P, 4, T], BF16)
                nc.sync.dma_start(qT_t.ap(), qT_sb[:])
                nc.sync.dma_start(kT_t.ap(), kT_sb[:])
                nc.sync.dma_start(v_t.ap(), v_sb[:])
                nc.sync.dma_start(at_t.ap(), attnT_sb[:])

    nc.finalize()  # Bacc: runs dce/alloc_regs/codegen passes
    return nc


_NC_CACHE = {}


def _get_nc(T):
    if T not in _NC_CACHE:
        _NC_CACHE[T] = build_nc(T)
    return _NC_CACHE[T]


def make_in_maps(x, Wq, Wk, Wv, Wo):
    bf = ml_dtypes.bfloat16
    in_maps = []
    for c in range(N_CORES):
        b, g = divmod(c, 2)
        gs = slice(g * D_LOCAL, (g + 1) * D_LOCAL)
        in_maps.append({
            "xT": np.ascontiguousarray(x[b].T).astype(bf),
            "wqT": np.ascontiguousarray(Wq[gs, :].T).astype(bf),
            "wkT": np.ascontiguousarray(Wk[gs, :].T).astype(bf),
            "wvT": np.ascontiguousarray(Wv[gs, :].T).astype(bf),
            "woT": np.ascontiguousarray(Wo[:, gs].T).astype(bf),
        })
    return in_maps


def assemble_out(outs, B, T, D):
    """Stitch per-core bf16 chunked-RS outputs into [B, T, D] f32.

    RS over pair [even, odd] splits each chunk's rows in half: the even
    core holds the first n*64 rows of the chunk, the odd core the rest.
    """
    y = np.empty((B, T, D), np.float32)
    for b in range(B):
        ev, od = outs[2 * b], outs[2 * b + 1]
        for c, (s, n) in enumerate(CHUNKS):
            base = s * 128
            half = n * 64
            y[b, base:base + half] = ev[f"out{c}"].astype(np.float32)
            y[b, base + half:base + 2 * half] = od[f"out{c}"].astype(
                np.float32)
    return y


# test harness hook: set RUN_OPTS["trace"]=True before calling kernel() to
# capture an NTFF profile; the BassKernelResults lands in RUN_OPTS["last"].
RUN_OPTS = {"trace": False, "tmpdir": None, "last": None}


def kernel(x, Wq, Wk, Wv, Wo):
    x = np.asarray(x, dtype=np.float32)
    B, T, D = x.shape
    nc = _get_nc(T)
    in_maps = make_in_maps(np.asarray(x), np.asarray(Wq), np.asarray(Wk),
                           np.asarray(Wv), np.asarray(Wo))
    res = run_bass_kernel_spmd(
        nc, in_maps, core_ids=list(range(N_CORES)),
        trace=RUN_OPTS["trace"], tmpdir=RUN_OPTS["tmpdir"],
    )
    RUN_OPTS["last"] = res
    return assemble_out(res.results, B, T, D)
